# revision 2
# baseline (speedup 1.0000x reference)
"""ChebyKAN layer on 8 Trainium2 NeuronCores (data-parallel over batch).

Computation:  out[b,o] = sum_{i,d} T_d(tanh(x)[b,i]) * C[i,o,d]
  - batch 32768 sharded 8 ways (4096 rows/core), coefficients replicated.
  - d=0 (T_0 == 1) is folded into a bias added at PSUM eviction.

Two kernel layouts, selected by CHEBY_LAYOUT:
  orig: cheby tiles [i,b] are the stationary matmul operand, C chunks [i,o]
        moving; out[b,o] accumulates in PSUM. One weight load per matmul.
  cs  : C chunks [i, o-slice] are stationary, cheby tiles [i,b] moving;
        outT[o,b] accumulates in PSUM (transposed back on host). Each weight
        load feeds 2 N=512 matmuls (b-groups of 1024, 4 o-slices x 2
        sub-blocks = 8 PSUM banks), halving LDWEIGHTS overhead.
"""

import os
from functools import lru_cache

import numpy as np
import ml_dtypes

import concourse.bass as bass
import concourse.mybir as mybir
import concourse.tile as tile
from concourse import bacc
from concourse.bass_utils import run_bass_kernel_spmd

N_CORES = 8
BATCH, IN_F, OUT_F, DEG = 32768, 512, 512, 8
B_LOC = BATCH // N_CORES  # 4096
P = 128
N_ICHUNK = IN_F // P  # 4
N_KCHUNK = DEG * N_ICHUNK  # 32 (d=0 handled as a bias add at copy-out)
N_OSLICE = OUT_F // P  # 4

MM_DT_NAME = os.environ.get("CHEBY_MM_DT", "f16")
_DT = {
    "bf16": (mybir.dt.bfloat16, ml_dtypes.bfloat16),
    "f16": (mybir.dt.float16, np.float16),
    "f32": (mybir.dt.float32, np.float32),
    "f32r": (mybir.dt.float32r, np.float32),
}
MM_DT, MM_NP = _DT[MM_DT_NAME]
LAYOUT = os.environ.get("CHEBY_LAYOUT", "cs")
# block of batch columns processed per iteration (SBUF-resident cheby tiles)
if LAYOUT == "cs":
    BBLK = 1024
else:
    BBLK = 512 if MM_DT_NAME in ("bf16", "f16") else 256
# 1 = single K=128 matmul per chunk; 2 = two concurrent K=64 row-group tiles
KSPLIT = int(os.environ.get("CHEBY_KSPLIT", "1"))
# coefficients scaled up on host so fp16 C stays normal; undone at copy-out
C_SCALE = 1024.0 if MM_DT_NAME == "f16" else 1.0


def _build_kernel(reps=1):
    f32 = mybir.dt.float32
    nc = bacc.Bacc(
        "TRN2",
        target_bir_lowering=False,
        debug=False,
        num_devices=N_CORES,
    )
    import contextlib

    xT = nc.declare_dram_parameter("xT", [IN_F, B_LOC], f32, isOutput=False)
    cw = nc.declare_dram_parameter("Cw", [N_KCHUNK * P, OUT_F], MM_DT, isOutput=False)
    xT_ap = xT[:, :].rearrange("(c p) b -> p c b", p=P)  # [128, 4, B_LOC]
    cw_ap = cw[:, :].rearrange("(k p) o -> p k o", p=P)  # [128, 32, 512]

    if LAYOUT == "cs":
        # bias broadcast tile: [p, os, b] with value bias[os*128+p]
        bias = nc.declare_dram_parameter("biasB", [P, N_OSLICE * 512], f32,
                                         isOutput=False)
        out = nc.declare_dram_parameter("outT", [OUT_F, B_LOC], f32, isOutput=True)
        out_ap = out[:, :].rearrange("(s p) b -> p s b", p=P)  # [128, 4, B_LOC]
        with tile.TileContext(nc) as tc:
            with (
                tc.tile_pool(name="const", bufs=1) as const_pool,
                tc.tile_pool(name="xin", bufs=3) as xin_pool,
                tc.tile_pool(name="tf32", bufs=2) as f32_pool,
                tc.tile_pool(name="cheb", bufs=6) as cheb_pool,
                tc.tile_pool(name="ot", bufs=4) as out_pool,
                tc.tile_pool(name="ps", bufs=8, space="PSUM") as psum_pool,
            ):
                c_tile = const_pool.tile([P, N_KCHUNK, OUT_F], MM_DT)
                nsplit = 4
                per = (N_KCHUNK + nsplit - 1) // nsplit
                for s in range(nsplit):
                    k0, k1 = s * per, min((s + 1) * per, N_KCHUNK)
                    nc.gpsimd.dma_start(out=c_tile[:, k0:k1, :], in_=cw_ap[:, k0:k1, :])
                b_tile = const_pool.tile([P, N_OSLICE, 512], f32)
                nc.gpsimd.dma_start(
                    out=b_tile[:, :, :],
                    in_=bias[:, :].rearrange("p (s b) -> p s b", s=N_OSLICE),
                )
                rep_ctx = (
                    tc.For_i(
                        0, reps, 1,
                        hint_engines=(
                            mybir.EngineType.PE,
                            mybir.EngineType.Activation,
                            mybir.EngineType.DVE,
                        ),
                    )
                    if reps > 1
                    else contextlib.nullcontext()
                )
                with rep_ctx:
                    _kernel_body_cs(nc, tc, xT_ap, c_tile, b_tile, out_ap,
                                    xin_pool, f32_pool, cheb_pool, out_pool,
                                    psum_pool)
    else:
        bias = nc.declare_dram_parameter("bias", [1, OUT_F], f32, isOutput=False)
        out = nc.declare_dram_parameter("out", [B_LOC, OUT_F], f32, isOutput=True)
        with tile.TileContext(nc) as tc:
            with (
                tc.tile_pool(name="const", bufs=1) as const_pool,
                tc.tile_pool(name="xin", bufs=3) as xin_pool,
                tc.tile_pool(name="tf32", bufs=1) as f32_pool,
                tc.tile_pool(name="cheb", bufs=2) as cheb_pool,
                tc.tile_pool(name="ot", bufs=4) as out_pool,
                tc.tile_pool(name="ps", bufs=6 // KSPLIT, space="PSUM") as psum_pool,
            ):
                c_tile = const_pool.tile([P, N_KCHUNK, OUT_F], MM_DT)
                nsplit = 4
                per = (N_KCHUNK + nsplit - 1) // nsplit
                for s in range(nsplit):
                    k0, k1 = s * per, min((s + 1) * per, N_KCHUNK)
                    nc.gpsimd.dma_start(out=c_tile[:, k0:k1, :], in_=cw_ap[:, k0:k1, :])
                # bias row (the folded d=0 term) broadcast to all 128 partitions
                b_tile = const_pool.tile([P, OUT_F], f32)
                bias_ap = bias[:, :]
                bias_bcast = bass.AP(
                    tensor=bias_ap.tensor,
                    offset=bias_ap.offset,
                    ap=[[0, P], bias_ap.ap[1]],
                )
                nc.gpsimd.dma_start(out=b_tile[:, :], in_=bias_bcast)

                rep_ctx = (
                    tc.For_i(
                        0, reps, 1,
                        hint_engines=(
                            mybir.EngineType.PE,
                            mybir.EngineType.Activation,
                            mybir.EngineType.DVE,
                        ),
                    )
                    if reps > 1
                    else contextlib.nullcontext()
                )
                with rep_ctx:
                    _kernel_body(nc, tc, xT_ap, c_tile, b_tile, out,
                                 xin_pool, f32_pool, cheb_pool, out_pool,
                                 psum_pool)
    nc.compile()
    return nc


def _cheby_ops(nc):
    MULT = mybir.AluOpType.mult

    def stt(o, a, b):  # o = 2*a*b
        nc.vector.scalar_tensor_tensor(
            out=o, in0=a, scalar=2.0, in1=b, op0=MULT, op1=MULT
        )

    def sub1(o):  # o -= 1
        nc.vector.tensor_scalar(
            out=o, in0=o, scalar1=1.0, scalar2=None,
            op0=mybir.AluOpType.subtract,
        )

    return stt, sub1


def _kernel_body_cs(nc, tc, xT_ap, c_tile, b_tile, out_ap,
                    xin_pool, f32_pool, cheb_pool, out_pool, psum_pool):
    """C-stationary layout: psum[o-slice, b] accumulates over the 32 k-chunks;
    each LDWEIGHTS of C[k, os] feeds 2 N=512 matmuls (the two 512-col halves
    of the 1024-col b-group). Chunk order is ichunk-major to match the cheby
    production order, keeping PE fed from the first chain on."""
    f32 = mybir.dt.float32
    ACT_F = mybir.ActivationFunctionType
    MULT = mybir.AluOpType.mult
    stt, sub1 = _cheby_ops(nc)
    NBB = BBLK // 512  # 2

    for blk in range(B_LOC // BBLK):
        b0 = blk * BBLK
        tb_tiles = []
        for ic in range(N_ICHUNK):
            x_p = xin_pool.tile([P, BBLK], f32)
            nc.sync.dma_start(out=x_p[:, :], in_=xT_ap[:, ic, b0 : b0 + BBLK])
            tfs = f32_pool.tile([P, 4, BBLK], f32)
            t1, t2, t3, t4 = (tfs[:, j, :] for j in range(4))
            tb = cheb_pool.tile([P, DEG, BBLK], MM_DT)
            nc.scalar.activation(out=t1, in_=x_p[:, :], func=ACT_F.Tanh)
            nc.scalar.copy(out=tb[:, 0, :], in_=t1)
            stt(t2, t1, t1); sub1(t2)
            nc.scalar.copy(out=tb[:, 1, :], in_=t2)
            stt(t3, t2, t1); nc.vector.tensor_sub(t3, t3, t1)
            nc.scalar.copy(out=tb[:, 2, :], in_=t3)
            stt(t4, t2, t2); sub1(t4)
            nc.scalar.copy(out=tb[:, 3, :], in_=t4)
            b1, b2, b3, b4 = (tb[:, j, :] for j in range(4))
            b5, b6, b7, b8 = (tb[:, j, :] for j in range(4, 8))
            stt(b5, b3, b2); nc.vector.tensor_sub(b5, b5, b1)
            stt(b6, b3, b3); sub1(b6)
            stt(b7, b4, b3); nc.vector.tensor_sub(b7, b7, b1)
            stt(b8, b4, b4); sub1(b8)
            tb_tiles.append(tb)

        ps = [
            psum_pool.tile([P, 512], f32, space="PSUM",
                           tag=f"ps{os}_{bb}", name=f"ps{os}_{bb}")
            for os in range(N_OSLICE)
            for bb in range(NBB)
        ]
        for ic in range(N_ICHUNK):
            for j in range(DEG):
                k = j * N_ICHUNK + ic
                first = ic == 0 and j == 0
                last = ic == N_ICHUNK - 1 and j == DEG - 1
                for os in range(N_OSLICE):
                    for bb in range(NBB):
                        nc.tensor.matmul(
                            ps[os * NBB + bb][:, :],
                            c_tile[:, k, os * P : (os + 1) * P],
                            tb_tiles[ic][:, j, bb * 512 : (bb + 1) * 512],
                            start=first,
                            stop=last,
                        )
        for os in range(N_OSLICE):
            for bb in range(NBB):
                o_tile = out_pool.tile([P, 512], f32)
                # out = psum / C_SCALE + bias   (bias = sum_i C[i,:,0])
                nc.vector.scalar_tensor_tensor(
                    out=o_tile[:, :],
                    in0=ps[os * NBB + bb][:, :],
                    scalar=1.0 / C_SCALE,
                    in1=b_tile[:, os, :],
                    op0=MULT,
                    op1=mybir.AluOpType.add,
                )
                c0 = b0 + bb * 512
                nc.sync.dma_start(
                    out=out_ap[:, os, c0 : c0 + 512], in_=o_tile[:, :]
                )


def _kernel_body(nc, tc, xT_ap, c_tile, b_tile, out,
                 xin_pool, f32_pool, cheb_pool, out_pool, psum_pool):
    f32 = mybir.dt.float32
    MULT = mybir.AluOpType.mult
    ACT_F = mybir.ActivationFunctionType
    stt, sub1 = _cheby_ops(nc)

    for blk in range(B_LOC // BBLK):
        b0 = blk * BBLK
        x_in = xin_pool.tile([P, N_ICHUNK, BBLK], f32)
        nc.sync.dma_start(out=x_in[:, :, :], in_=xT_ap[:, :, b0 : b0 + BBLK])

        # Tf[:, j] = T_{j+1} in fp32 (j=0..3); Tb[:, j] = T_{j+1} in bf16 (j=0..7)
        Tf = f32_pool.tile([P, 4, N_ICHUNK, BBLK], f32)
        Tb = cheb_pool.tile([P, DEG, N_ICHUNK, BBLK], MM_DT)
        t1, t2, t3, t4 = (Tf[:, j, :, :] for j in range(4))
        nc.scalar.activation(out=t1, in_=x_in[:, :, :], func=ACT_F.Tanh)
        # fp32 chain: T2=2T1^2-1, T3=2T2T1-T1, T4=2T2^2-1
        stt(t2, t1, t1); sub1(t2)
        stt(t3, t2, t1); nc.vector.tensor_sub(t3, t3, t1)
        stt(t4, t2, t2); sub1(t4)
        # one-time rounding to bf16 on the scalar engine
        for j in range(4):
            nc.scalar.copy(out=Tb[:, j, :, :], in_=Tf[:, j, :, :])
        b1, b2, b3, b4 = (Tb[:, j, :, :] for j in range(4))
        b5, b6, b7, b8 = (Tb[:, j, :, :] for j in range(4, 8))
        # bf16 products: T5=2T3T2-T1, T6=2T3^2-1, T7=2T4T3-T1, T8=2T4^2-1
        stt(b5, b3, b2); nc.vector.tensor_sub(b5, b5, b1)
        stt(b6, b3, b3); sub1(b6)
        stt(b7, b4, b3); nc.vector.tensor_sub(b7, b7, b1)
        stt(b8, b4, b4); sub1(b8)

        for bt in range(BBLK // P):
            H = P // KSPLIT
            halves = [(h, h * H) for h in range(KSPLIT)]
            ps = [
                psum_pool.tile(
                    [P, OUT_F], f32, space="PSUM", tag=f"ps{h}", name=f"ps{h}"
                )
                for h in range(KSPLIT)
            ]
            bsl = slice(bt * P, (bt + 1) * P)
            for j in range(DEG):
                for c in range(N_ICHUNK):
                    k = j * N_ICHUNK + c
                    for h, lo in halves:
                        nc.tensor.matmul(
                            ps[h][:, :],
                            Tb[lo : lo + H, j, c, bsl],
                            c_tile[lo : lo + H, k, :],
                            start=(k == 0),
                            stop=(k == N_KCHUNK - 1),
                        )
            o_tile = out_pool.tile([P, OUT_F], f32)
            row = b0 + bt * P
            acc = ps[0][:, :]
            if KSPLIT > 1:
                half_sb = out_pool.tile([P, OUT_F], f32, tag="halfsb")
                nc.scalar.copy(out=half_sb[:, :], in_=ps[0][:, :])
                for h in range(1, KSPLIT - 1):
                    nc.vector.tensor_add(half_sb[:, :], half_sb[:, :], ps[h][:, :])
                nc.vector.tensor_add(half_sb[:, :], half_sb[:, :], ps[KSPLIT - 1][:, :])
                acc = half_sb[:, :]
            # out = psum / C_SCALE + bias   (bias = sum_i C[i,:,0], the d=0 term)
            nc.vector.scalar_tensor_tensor(
                out=o_tile[:, :],
                in0=acc,
                scalar=1.0 / C_SCALE,
                in1=b_tile[:, :],
                op0=MULT,
                op1=mybir.AluOpType.add,
            )
            nc.sync.dma_start(out=out[row : row + P, :], in_=o_tile[:, :])


@lru_cache(maxsize=4)
def _get_nc(reps=1):
    return _build_kernel(reps)


class Runner:
    """Persistent jitted runner mirroring bass2jax.run_bass_via_pjrt, reusable
    across calls (single jit cache entry) so repeated executions can be timed
    back-to-back without recompilation or host round-trips per call."""

    def __init__(self, nc):
        import jax
        import jax.numpy as jnp
        from jax.sharding import Mesh, PartitionSpec
        from jax.experimental.shard_map import shard_map
        from concourse import bass2jax
        from concourse import mybir as _mybir

        bass2jax.install_neuronx_cc_hook()
        self.jax = jax
        self.nc = nc
        partition_name = (
            nc.partition_id_tensor.name if nc.partition_id_tensor else None
        )
        in_names, out_names, out_avals = [], [], []
        for alloc in nc.m.functions[0].allocations:
            if not isinstance(alloc, _mybir.MemoryLocationSet):
                continue
            name = alloc.memorylocations[0].name
            if alloc.kind == "ExternalInput":
                if name != partition_name:
                    in_names.append(name)
            elif alloc.kind == "ExternalOutput":
                out_names.append(name)
                out_avals.append(
                    jax.core.ShapedArray(
                        tuple(alloc.tensor_shape), _mybir.dt.np(alloc.dtype)
                    )
                )
        self.in_names = list(in_names)
        self.out_names = out_names
        self.out_avals = out_avals
        n_params = len(in_names)
        all_names = in_names + out_names
        if partition_name is not None:
            all_names = all_names + [partition_name]

        def _body(*args):
            operands = list(args)
            if partition_name is not None:
                operands.append(bass2jax.partition_id_tensor())
            return tuple(
                bass2jax._bass_exec_p.bind(
                    *operands,
                    out_avals=tuple(out_avals),
                    in_names=tuple(all_names),
                    out_names=tuple(out_names),
                    lowering_input_output_aliases=(),
                    sim_require_finite=True,
                    sim_require_nnan=True,
                    nc=nc,
                )
            )

        devices = jax.devices()[:N_CORES]
        self.mesh = Mesh(np.asarray(devices), ("core",))
        in_specs = (PartitionSpec("core"),) * (n_params + len(out_names))
        out_specs = (PartitionSpec("core"),) * len(out_names)
        self.fn = jax.jit(
            shard_map(
                _body,
                mesh=self.mesh,
                in_specs=in_specs,
                out_specs=out_specs,
                check_rep=False,
            ),
            keep_unused=True,
        )

    def put_inputs(self, in_maps):
        import jax
        from jax.sharding import NamedSharding, PartitionSpec

        concat = [
            np.concatenate([np.asarray(m[name]) for m in in_maps], axis=0)
            for name in self.in_names
        ]
        for aval in self.out_avals:
            concat.append(
                np.zeros((N_CORES * aval.shape[0], *aval.shape[1:]), aval.dtype)
            )
        sh = NamedSharding(self.mesh, PartitionSpec("core"))
        return [jax.device_put(a, sh) for a in concat]

    def __call__(self, dev_inputs):
        return self.fn(*dev_inputs)

    def run_np(self, in_maps):
        outs = self(self.put_inputs(in_maps))
        return [
            {
                name: np.asarray(outs[i]).reshape(N_CORES, *self.out_avals[i].shape)[c]
                for i, name in enumerate(self.out_names)
            }
            for c in range(N_CORES)
        ]


def _prep_inputs(x: np.ndarray, coefficients: np.ndarray):
    x = np.asarray(x, dtype=np.float32)
    coefficients = np.asarray(coefficients, dtype=np.float32)
    # chunk k = j*4+c is degree j+1, i-chunk c, laid out [i within chunk, o];
    # the d=0 term (T_0 == 1) reduces to a bias row added at copy-out.
    c_perm = np.transpose(coefficients, (2, 0, 1))  # (d, i, o)
    bias = np.ascontiguousarray(c_perm[0].sum(axis=0, dtype=np.float64))
    bias = bias.astype(np.float32)
    c_main = c_perm[1:].reshape(N_KCHUNK * P, OUT_F) * C_SCALE
    c_all = np.ascontiguousarray(c_main).astype(MM_NP)

    if LAYOUT == "cs":
        # biasB[p, os*512 + b] = bias[os*128 + p]  (broadcast along b)
        bias_t = bias.reshape(N_OSLICE, P).T  # [128, 4]
        bias_b = np.ascontiguousarray(
            np.broadcast_to(bias_t[:, :, None], (P, N_OSLICE, 512))
        ).reshape(P, N_OSLICE * 512)
    else:
        bias_b = bias.reshape(1, OUT_F)

    in_maps = []
    for core in range(N_CORES):
        shard = x[core * B_LOC : (core + 1) * B_LOC]  # (4096, 512)
        xt = np.ascontiguousarray(shard.T)  # (512, 4096)
        m = {"xT": xt, "Cw": c_all}
        m["biasB" if LAYOUT == "cs" else "bias"] = bias_b
        in_maps.append(m)
    return in_maps


@lru_cache(maxsize=4)
def _get_runner(reps=1):
    return Runner(_get_nc(reps))


def run_sharded(x, coefficients):
    """Run the 8-core kernel; returns the full (32768, 512) float32 output."""
    in_maps = _prep_inputs(x, coefficients)
    runner = _get_runner()
    results = runner.run_np(in_maps)
    if LAYOUT == "cs":
        parts = [np.asarray(results[i]["outT"]).T for i in range(N_CORES)]
    else:
        parts = [np.asarray(results[i]["out"]) for i in range(N_CORES)]
    return np.concatenate(parts, axis=0).astype(np.float32)


def _time_runner(runner, dev_in, iters):
    import time

    outs = runner(dev_in)  # warm up
    outs[0].block_until_ready()
    times = []
    for _ in range(iters):
        t0 = time.perf_counter()
        outs = runner(dev_in)
        outs[0].block_until_ready()
        times.append((time.perf_counter() - t0) * 1e9)
    return times


def bench(x, coefficients, iters=12, rep_a=3, rep_b=83):
    """Estimate per-invocation HW time from the slope between two on-device
    repeat counts (fixed ~66-107ms axon RPC overhead cancels). Interleaved
    rounds + median to reject the bimodal RPC jitter. Returns
    (slope_ns, times_a, times_b)."""
    in_maps = _prep_inputs(x, coefficients)
    ra, rb = _get_runner(rep_a), _get_runner(rep_b)
    dev_a = ra.put_inputs(in_maps)
    dev_b = rb.put_inputs(in_maps)
    ta, tb = [], []
    for _ in range(3):
        ta += _time_runner(ra, dev_a, iters // 3 + 1)
        tb += _time_runner(rb, dev_b, iters // 3 + 1)
    med = lambda t: sorted(t)[len(t) // 2]
    slope = (med(tb) - med(ta)) / (rep_b - rep_a)
    return slope, ta, tb


def kernel(x, coefficients):
    return run_sharded(x, coefficients)


# revision 3
# speedup vs baseline: 1.3022x; 1.3022x over previous
"""ChebyKAN layer on 8 Trainium2 NeuronCores (data-parallel over batch).

Computation:  out[b,o] = sum_{i,d} T_d(tanh(x)[b,i]) * C[i,o,d]
  - batch 32768 sharded 8 ways (4096 rows/core), coefficients replicated.
  - d=0 (T_0 == 1) is folded into a bias added at PSUM eviction.

Two kernel layouts, selected by CHEBY_LAYOUT:
  orig: cheby tiles [i,b] are the stationary matmul operand, C chunks [i,o]
        moving; out[b,o] accumulates in PSUM. One weight load per matmul.
  cs  : C chunks [i, o-slice] are stationary, cheby tiles [i,b] moving;
        outT[o,b] accumulates in PSUM (transposed back on host). Each weight
        load feeds 2 N=512 matmuls (b-groups of 1024, 4 o-slices x 2
        sub-blocks = 8 PSUM banks), halving LDWEIGHTS overhead.
"""

import os
from functools import lru_cache

import numpy as np
import ml_dtypes

import concourse.bass as bass
import concourse.mybir as mybir
import concourse.tile as tile
from concourse import bacc
from concourse.bass_utils import run_bass_kernel_spmd

N_CORES = 8
BATCH, IN_F, OUT_F, DEG = 32768, 512, 512, 8
B_LOC = BATCH // N_CORES  # 4096
P = 128
N_ICHUNK = IN_F // P  # 4
N_KCHUNK = DEG * N_ICHUNK  # 32 (d=0 handled as a bias add at copy-out)
N_OSLICE = OUT_F // P  # 4

MM_DT_NAME = os.environ.get("CHEBY_MM_DT", "f16")
_DT = {
    "bf16": (mybir.dt.bfloat16, ml_dtypes.bfloat16),
    "f16": (mybir.dt.float16, np.float16),
    "f32": (mybir.dt.float32, np.float32),
    "f32r": (mybir.dt.float32r, np.float32),
}
MM_DT, MM_NP = _DT[MM_DT_NAME]
LAYOUT = os.environ.get("CHEBY_LAYOUT", "cs")
# block of batch columns processed per iteration (SBUF-resident cheby tiles)
if LAYOUT == "cs":
    BBLK = 1024
else:
    BBLK = 512 if MM_DT_NAME in ("bf16", "f16") else 256
# 1 = single K=128 matmul per chunk; 2 = two concurrent K=64 row-group tiles
KSPLIT = int(os.environ.get("CHEBY_KSPLIT", "1"))
# coefficients scaled up on host so fp16 C stays normal; undone at copy-out
C_SCALE = 1024.0 if MM_DT_NAME == "f16" else 1.0


def _build_kernel(reps=1):
    f32 = mybir.dt.float32
    nc = bacc.Bacc(
        "TRN2",
        target_bir_lowering=False,
        debug=False,
        num_devices=N_CORES,
    )
    import contextlib

    xT = nc.declare_dram_parameter("xT", [IN_F, B_LOC], f32, isOutput=False)
    cw = nc.declare_dram_parameter("Cw", [N_KCHUNK * P, OUT_F], MM_DT, isOutput=False)
    xT_ap = xT[:, :].rearrange("(c p) b -> p c b", p=P)  # [128, 4, B_LOC]
    cw_ap = cw[:, :].rearrange("(k p) o -> p k o", p=P)  # [128, 32, 512]

    if LAYOUT == "cs":
        # bias broadcast tile: [p, os, b] with value bias[os*128+p]
        bias = nc.declare_dram_parameter("biasB", [P, N_OSLICE * 512], f32,
                                         isOutput=False)
        out = nc.declare_dram_parameter("outT", [OUT_F, B_LOC], f32, isOutput=True)
        out_ap = out[:, :].rearrange("(s p) b -> p s b", p=P)  # [128, 4, B_LOC]
        with tile.TileContext(nc) as tc:
            with (
                tc.tile_pool(name="const", bufs=1) as const_pool,
                tc.tile_pool(name="xin", bufs=3) as xin_pool,
                tc.tile_pool(name="tf32", bufs=2) as f32_pool,
                tc.tile_pool(name="cheb", bufs=6) as cheb_pool,
                tc.tile_pool(name="ot", bufs=4) as out_pool,
                tc.tile_pool(name="ps", bufs=1, space="PSUM") as psum_pool,
            ):
                c_tile = const_pool.tile([P, N_KCHUNK, OUT_F], MM_DT)
                nsplit = 4
                per = (N_KCHUNK + nsplit - 1) // nsplit
                for s in range(nsplit):
                    k0, k1 = s * per, min((s + 1) * per, N_KCHUNK)
                    nc.gpsimd.dma_start(out=c_tile[:, k0:k1, :], in_=cw_ap[:, k0:k1, :])
                b_tile = const_pool.tile([P, N_OSLICE, 512], f32)
                nc.gpsimd.dma_start(
                    out=b_tile[:, :, :],
                    in_=bias[:, :].rearrange("p (s b) -> p s b", s=N_OSLICE),
                )
                rep_ctx = (
                    tc.For_i(
                        0, reps, 1,
                        hint_engines=(
                            mybir.EngineType.PE,
                            mybir.EngineType.Activation,
                            mybir.EngineType.DVE,
                        ),
                    )
                    if reps > 1
                    else contextlib.nullcontext()
                )
                with rep_ctx:
                    _kernel_body_cs(nc, tc, xT_ap, c_tile, b_tile, out_ap,
                                    xin_pool, f32_pool, cheb_pool, out_pool,
                                    psum_pool)
    else:
        bias = nc.declare_dram_parameter("bias", [1, OUT_F], f32, isOutput=False)
        out = nc.declare_dram_parameter("out", [B_LOC, OUT_F], f32, isOutput=True)
        with tile.TileContext(nc) as tc:
            with (
                tc.tile_pool(name="const", bufs=1) as const_pool,
                tc.tile_pool(name="xin", bufs=3) as xin_pool,
                tc.tile_pool(name="tf32", bufs=1) as f32_pool,
                tc.tile_pool(name="cheb", bufs=2) as cheb_pool,
                tc.tile_pool(name="ot", bufs=4) as out_pool,
                tc.tile_pool(name="ps", bufs=6 // KSPLIT, space="PSUM") as psum_pool,
            ):
                c_tile = const_pool.tile([P, N_KCHUNK, OUT_F], MM_DT)
                nsplit = 4
                per = (N_KCHUNK + nsplit - 1) // nsplit
                for s in range(nsplit):
                    k0, k1 = s * per, min((s + 1) * per, N_KCHUNK)
                    nc.gpsimd.dma_start(out=c_tile[:, k0:k1, :], in_=cw_ap[:, k0:k1, :])
                # bias row (the folded d=0 term) broadcast to all 128 partitions
                b_tile = const_pool.tile([P, OUT_F], f32)
                bias_ap = bias[:, :]
                bias_bcast = bass.AP(
                    tensor=bias_ap.tensor,
                    offset=bias_ap.offset,
                    ap=[[0, P], bias_ap.ap[1]],
                )
                nc.gpsimd.dma_start(out=b_tile[:, :], in_=bias_bcast)

                rep_ctx = (
                    tc.For_i(
                        0, reps, 1,
                        hint_engines=(
                            mybir.EngineType.PE,
                            mybir.EngineType.Activation,
                            mybir.EngineType.DVE,
                        ),
                    )
                    if reps > 1
                    else contextlib.nullcontext()
                )
                with rep_ctx:
                    _kernel_body(nc, tc, xT_ap, c_tile, b_tile, out,
                                 xin_pool, f32_pool, cheb_pool, out_pool,
                                 psum_pool)
    nc.compile()
    return nc


def _cheby_ops(nc):
    MULT = mybir.AluOpType.mult

    def stt(o, a, b):  # o = 2*a*b
        nc.vector.scalar_tensor_tensor(
            out=o, in0=a, scalar=2.0, in1=b, op0=MULT, op1=MULT
        )

    def sub1(o):  # o -= 1
        nc.vector.tensor_scalar(
            out=o, in0=o, scalar1=1.0, scalar2=None,
            op0=mybir.AluOpType.subtract,
        )

    return stt, sub1


def _kernel_body_cs(nc, tc, xT_ap, c_tile, b_tile, out_ap,
                    xin_pool, f32_pool, cheb_pool, out_pool, psum_pool):
    """C-stationary layout: psum[o-slice, b] accumulates over the 32 k-chunks;
    each LDWEIGHTS of C[k, os] feeds 2 N=512 matmuls (the two 512-col halves
    of the 1024-col b-group). Chunk order is ichunk-major to match the cheby
    production order, keeping PE fed from the first chain on."""
    f32 = mybir.dt.float32
    ACT_F = mybir.ActivationFunctionType
    MULT = mybir.AluOpType.mult
    stt, sub1 = _cheby_ops(nc)
    NBB = BBLK // 512  # 2

    for blk in range(B_LOC // BBLK):
        b0 = blk * BBLK
        tb_tiles = []
        for ic in range(N_ICHUNK):
            x_p = xin_pool.tile([P, BBLK], f32)
            nc.sync.dma_start(out=x_p[:, :], in_=xT_ap[:, ic, b0 : b0 + BBLK])
            tfs = f32_pool.tile([P, 4, BBLK], f32)
            t1, t2, t3, t4 = (tfs[:, j, :] for j in range(4))
            tb = cheb_pool.tile([P, DEG, BBLK], MM_DT)
            nc.scalar.activation(out=t1, in_=x_p[:, :], func=ACT_F.Tanh)
            nc.scalar.copy(out=tb[:, 0, :], in_=t1)
            stt(t2, t1, t1); sub1(t2)
            nc.scalar.copy(out=tb[:, 1, :], in_=t2)
            stt(t3, t2, t1); nc.vector.tensor_sub(t3, t3, t1)
            nc.scalar.copy(out=tb[:, 2, :], in_=t3)
            stt(t4, t2, t2); sub1(t4)
            nc.scalar.copy(out=tb[:, 3, :], in_=t4)
            b1, b2, b3, b4 = (tb[:, j, :] for j in range(4))
            b5, b6, b7, b8 = (tb[:, j, :] for j in range(4, 8))
            stt(b5, b3, b2); nc.vector.tensor_sub(b5, b5, b1)
            stt(b6, b3, b3); sub1(b6)
            stt(b7, b4, b3); nc.vector.tensor_sub(b7, b7, b1)
            stt(b8, b4, b4); sub1(b8)
            tb_tiles.append(tb)

        ps = [
            psum_pool.tile([P, 512], f32, space="PSUM",
                           tag=f"ps{os}_{bb}", name=f"ps{os}_{bb}")
            for os in range(N_OSLICE)
            for bb in range(NBB)
        ]
        for ic in range(N_ICHUNK):
            for j in range(DEG):
                k = j * N_ICHUNK + ic
                first = ic == 0 and j == 0
                last = ic == N_ICHUNK - 1 and j == DEG - 1
                for os in range(N_OSLICE):
                    for bb in range(NBB):
                        nc.tensor.matmul(
                            ps[os * NBB + bb][:, :],
                            c_tile[:, k, os * P : (os + 1) * P],
                            tb_tiles[ic][:, j, bb * 512 : (bb + 1) * 512],
                            start=first,
                            stop=last,
                        )
        for os in range(N_OSLICE):
            for bb in range(NBB):
                o_tile = out_pool.tile([P, 512], f32)
                # out = psum / C_SCALE + bias   (bias = sum_i C[i,:,0])
                nc.vector.scalar_tensor_tensor(
                    out=o_tile[:, :],
                    in0=ps[os * NBB + bb][:, :],
                    scalar=1.0 / C_SCALE,
                    in1=b_tile[:, os, :],
                    op0=MULT,
                    op1=mybir.AluOpType.add,
                )
                c0 = b0 + bb * 512
                nc.sync.dma_start(
                    out=out_ap[:, os, c0 : c0 + 512], in_=o_tile[:, :]
                )


def _kernel_body(nc, tc, xT_ap, c_tile, b_tile, out,
                 xin_pool, f32_pool, cheb_pool, out_pool, psum_pool):
    f32 = mybir.dt.float32
    MULT = mybir.AluOpType.mult
    ACT_F = mybir.ActivationFunctionType
    stt, sub1 = _cheby_ops(nc)

    for blk in range(B_LOC // BBLK):
        b0 = blk * BBLK
        x_in = xin_pool.tile([P, N_ICHUNK, BBLK], f32)
        nc.sync.dma_start(out=x_in[:, :, :], in_=xT_ap[:, :, b0 : b0 + BBLK])

        # Tf[:, j] = T_{j+1} in fp32 (j=0..3); Tb[:, j] = T_{j+1} in bf16 (j=0..7)
        Tf = f32_pool.tile([P, 4, N_ICHUNK, BBLK], f32)
        Tb = cheb_pool.tile([P, DEG, N_ICHUNK, BBLK], MM_DT)
        t1, t2, t3, t4 = (Tf[:, j, :, :] for j in range(4))
        nc.scalar.activation(out=t1, in_=x_in[:, :, :], func=ACT_F.Tanh)
        # fp32 chain: T2=2T1^2-1, T3=2T2T1-T1, T4=2T2^2-1
        stt(t2, t1, t1); sub1(t2)
        stt(t3, t2, t1); nc.vector.tensor_sub(t3, t3, t1)
        stt(t4, t2, t2); sub1(t4)
        # one-time rounding to bf16 on the scalar engine
        for j in range(4):
            nc.scalar.copy(out=Tb[:, j, :, :], in_=Tf[:, j, :, :])
        b1, b2, b3, b4 = (Tb[:, j, :, :] for j in range(4))
        b5, b6, b7, b8 = (Tb[:, j, :, :] for j in range(4, 8))
        # bf16 products: T5=2T3T2-T1, T6=2T3^2-1, T7=2T4T3-T1, T8=2T4^2-1
        stt(b5, b3, b2); nc.vector.tensor_sub(b5, b5, b1)
        stt(b6, b3, b3); sub1(b6)
        stt(b7, b4, b3); nc.vector.tensor_sub(b7, b7, b1)
        stt(b8, b4, b4); sub1(b8)

        for bt in range(BBLK // P):
            H = P // KSPLIT
            halves = [(h, h * H) for h in range(KSPLIT)]
            ps = [
                psum_pool.tile(
                    [P, OUT_F], f32, space="PSUM", tag=f"ps{h}", name=f"ps{h}"
                )
                for h in range(KSPLIT)
            ]
            bsl = slice(bt * P, (bt + 1) * P)
            for j in range(DEG):
                for c in range(N_ICHUNK):
                    k = j * N_ICHUNK + c
                    for h, lo in halves:
                        nc.tensor.matmul(
                            ps[h][:, :],
                            Tb[lo : lo + H, j, c, bsl],
                            c_tile[lo : lo + H, k, :],
                            start=(k == 0),
                            stop=(k == N_KCHUNK - 1),
                        )
            o_tile = out_pool.tile([P, OUT_F], f32)
            row = b0 + bt * P
            acc = ps[0][:, :]
            if KSPLIT > 1:
                half_sb = out_pool.tile([P, OUT_F], f32, tag="halfsb")
                nc.scalar.copy(out=half_sb[:, :], in_=ps[0][:, :])
                for h in range(1, KSPLIT - 1):
                    nc.vector.tensor_add(half_sb[:, :], half_sb[:, :], ps[h][:, :])
                nc.vector.tensor_add(half_sb[:, :], half_sb[:, :], ps[KSPLIT - 1][:, :])
                acc = half_sb[:, :]
            # out = psum / C_SCALE + bias   (bias = sum_i C[i,:,0], the d=0 term)
            nc.vector.scalar_tensor_tensor(
                out=o_tile[:, :],
                in0=acc,
                scalar=1.0 / C_SCALE,
                in1=b_tile[:, :],
                op0=MULT,
                op1=mybir.AluOpType.add,
            )
            nc.sync.dma_start(out=out[row : row + P, :], in_=o_tile[:, :])


@lru_cache(maxsize=4)
def _get_nc(reps=1):
    return _build_kernel(reps)


class Runner:
    """Persistent jitted runner mirroring bass2jax.run_bass_via_pjrt, reusable
    across calls (single jit cache entry) so repeated executions can be timed
    back-to-back without recompilation or host round-trips per call."""

    def __init__(self, nc):
        import jax
        import jax.numpy as jnp
        from jax.sharding import Mesh, PartitionSpec
        from jax.experimental.shard_map import shard_map
        from concourse import bass2jax
        from concourse import mybir as _mybir

        bass2jax.install_neuronx_cc_hook()
        self.jax = jax
        self.nc = nc
        partition_name = (
            nc.partition_id_tensor.name if nc.partition_id_tensor else None
        )
        in_names, out_names, out_avals = [], [], []
        for alloc in nc.m.functions[0].allocations:
            if not isinstance(alloc, _mybir.MemoryLocationSet):
                continue
            name = alloc.memorylocations[0].name
            if alloc.kind == "ExternalInput":
                if name != partition_name:
                    in_names.append(name)
            elif alloc.kind == "ExternalOutput":
                out_names.append(name)
                out_avals.append(
                    jax.core.ShapedArray(
                        tuple(alloc.tensor_shape), _mybir.dt.np(alloc.dtype)
                    )
                )
        self.in_names = list(in_names)
        self.out_names = out_names
        self.out_avals = out_avals
        n_params = len(in_names)
        all_names = in_names + out_names
        if partition_name is not None:
            all_names = all_names + [partition_name]

        def _body(*args):
            operands = list(args)
            if partition_name is not None:
                operands.append(bass2jax.partition_id_tensor())
            return tuple(
                bass2jax._bass_exec_p.bind(
                    *operands,
                    out_avals=tuple(out_avals),
                    in_names=tuple(all_names),
                    out_names=tuple(out_names),
                    lowering_input_output_aliases=(),
                    sim_require_finite=True,
                    sim_require_nnan=True,
                    nc=nc,
                )
            )

        devices = jax.devices()[:N_CORES]
        self.mesh = Mesh(np.asarray(devices), ("core",))
        in_specs = (PartitionSpec("core"),) * (n_params + len(out_names))
        out_specs = (PartitionSpec("core"),) * len(out_names)
        self.fn = jax.jit(
            shard_map(
                _body,
                mesh=self.mesh,
                in_specs=in_specs,
                out_specs=out_specs,
                check_rep=False,
            ),
            keep_unused=True,
        )

    def put_inputs(self, in_maps):
        import jax
        from jax.sharding import NamedSharding, PartitionSpec

        concat = [
            np.concatenate([np.asarray(m[name]) for m in in_maps], axis=0)
            for name in self.in_names
        ]
        for aval in self.out_avals:
            concat.append(
                np.zeros((N_CORES * aval.shape[0], *aval.shape[1:]), aval.dtype)
            )
        sh = NamedSharding(self.mesh, PartitionSpec("core"))
        return [jax.device_put(a, sh) for a in concat]

    def __call__(self, dev_inputs):
        return self.fn(*dev_inputs)

    def run_np(self, in_maps):
        outs = self(self.put_inputs(in_maps))
        return [
            {
                name: np.asarray(outs[i]).reshape(N_CORES, *self.out_avals[i].shape)[c]
                for i, name in enumerate(self.out_names)
            }
            for c in range(N_CORES)
        ]


def _prep_inputs(x: np.ndarray, coefficients: np.ndarray):
    x = np.asarray(x, dtype=np.float32)
    coefficients = np.asarray(coefficients, dtype=np.float32)
    # chunk k = j*4+c is degree j+1, i-chunk c, laid out [i within chunk, o];
    # the d=0 term (T_0 == 1) reduces to a bias row added at copy-out.
    c_perm = np.transpose(coefficients, (2, 0, 1))  # (d, i, o)
    bias = np.ascontiguousarray(c_perm[0].sum(axis=0, dtype=np.float64))
    bias = bias.astype(np.float32)
    c_main = c_perm[1:].reshape(N_KCHUNK * P, OUT_F) * C_SCALE
    c_all = np.ascontiguousarray(c_main).astype(MM_NP)

    if LAYOUT == "cs":
        # biasB[p, os*512 + b] = bias[os*128 + p]  (broadcast along b)
        bias_t = bias.reshape(N_OSLICE, P).T  # [128, 4]
        bias_b = np.ascontiguousarray(
            np.broadcast_to(bias_t[:, :, None], (P, N_OSLICE, 512))
        ).reshape(P, N_OSLICE * 512)
    else:
        bias_b = bias.reshape(1, OUT_F)

    in_maps = []
    for core in range(N_CORES):
        shard = x[core * B_LOC : (core + 1) * B_LOC]  # (4096, 512)
        xt = np.ascontiguousarray(shard.T)  # (512, 4096)
        m = {"xT": xt, "Cw": c_all}
        m["biasB" if LAYOUT == "cs" else "bias"] = bias_b
        in_maps.append(m)
    return in_maps


@lru_cache(maxsize=4)
def _get_runner(reps=1):
    return Runner(_get_nc(reps))


def run_sharded(x, coefficients):
    """Run the 8-core kernel; returns the full (32768, 512) float32 output."""
    in_maps = _prep_inputs(x, coefficients)
    runner = _get_runner()
    results = runner.run_np(in_maps)
    if LAYOUT == "cs":
        parts = [np.asarray(results[i]["outT"]).T for i in range(N_CORES)]
    else:
        parts = [np.asarray(results[i]["out"]) for i in range(N_CORES)]
    return np.concatenate(parts, axis=0).astype(np.float32)


def _time_runner(runner, dev_in, iters):
    import time

    outs = runner(dev_in)  # warm up
    outs[0].block_until_ready()
    times = []
    for _ in range(iters):
        t0 = time.perf_counter()
        outs = runner(dev_in)
        outs[0].block_until_ready()
        times.append((time.perf_counter() - t0) * 1e9)
    return times


def bench(x, coefficients, iters=12, rep_a=3, rep_b=83):
    """Estimate per-invocation HW time from the slope between two on-device
    repeat counts (fixed ~66-107ms axon RPC overhead cancels). Interleaved
    rounds + median to reject the bimodal RPC jitter. Returns
    (slope_ns, times_a, times_b)."""
    in_maps = _prep_inputs(x, coefficients)
    ra, rb = _get_runner(rep_a), _get_runner(rep_b)
    dev_a = ra.put_inputs(in_maps)
    dev_b = rb.put_inputs(in_maps)
    ta, tb = [], []
    for _ in range(3):
        ta += _time_runner(ra, dev_a, iters // 3 + 1)
        tb += _time_runner(rb, dev_b, iters // 3 + 1)
    med = lambda t: sorted(t)[len(t) // 2]
    slope = (med(tb) - med(ta)) / (rep_b - rep_a)
    return slope, ta, tb


def kernel(x, coefficients):
    return run_sharded(x, coefficients)


# revision 5
# speedup vs baseline: 1.3746x; 1.0556x over previous
"""ChebyKAN layer on 8 Trainium2 NeuronCores (data-parallel over batch).

Computation:  out[b,o] = sum_{i,d} T_d(tanh(x)[b,i]) * C[i,o,d]
  - batch 32768 sharded 8 ways (4096 rows/core), coefficients replicated.
  - d=0 (T_0 == 1) is folded into a bias added at PSUM eviction.

Two kernel layouts, selected by CHEBY_LAYOUT:
  orig: cheby tiles [i,b] are the stationary matmul operand, C chunks [i,o]
        moving; out[b,o] accumulates in PSUM. One weight load per matmul.
  cs  : C chunks [i, o-slice] are stationary, cheby tiles [i,b] moving;
        outT[o,b] accumulates in PSUM (transposed back on host). Each weight
        load feeds 2 N=512 matmuls (b-groups of 1024, 4 o-slices x 2
        sub-blocks = 8 PSUM banks), halving LDWEIGHTS overhead.
"""

import os
from functools import lru_cache

import numpy as np
import ml_dtypes

import concourse.bass as bass
import concourse.mybir as mybir
import concourse.tile as tile
from concourse import bacc
from concourse.bass_utils import run_bass_kernel_spmd

N_CORES = 8
BATCH, IN_F, OUT_F, DEG = 32768, 512, 512, 8
B_LOC = BATCH // N_CORES  # 4096
P = 128
N_ICHUNK = IN_F // P  # 4
N_KCHUNK = DEG * N_ICHUNK  # 32 (d=0 handled as a bias add at copy-out)
N_OSLICE = OUT_F // P  # 4

MM_DT_NAME = os.environ.get("CHEBY_MM_DT", "f16")
_DT = {
    "bf16": (mybir.dt.bfloat16, ml_dtypes.bfloat16),
    "f16": (mybir.dt.float16, np.float16),
    "f32": (mybir.dt.float32, np.float32),
    "f32r": (mybir.dt.float32r, np.float32),
}
MM_DT, MM_NP = _DT[MM_DT_NAME]
LAYOUT = os.environ.get("CHEBY_LAYOUT", "cs")
# block of batch columns processed per iteration (SBUF-resident cheby tiles)
if LAYOUT == "cs":
    BBLK = 1024
else:
    BBLK = 512 if MM_DT_NAME in ("bf16", "f16") else 256
# 1 = single K=128 matmul per chunk; 2 = two concurrent K=64 row-group tiles
KSPLIT = int(os.environ.get("CHEBY_KSPLIT", "1"))
# coefficients scaled up on host so fp16 C stays normal; undone at copy-out
C_SCALE = 1024.0 if MM_DT_NAME == "f16" else 1.0


def _dedup_ldweights(nc):
    """Drop InstLdweights that reload the exact weights already resident in
    the PE array (the compile pipeline splits every matmul into
    Ldweights+Matmult and does not dedup consecutive identical loads).
    Matmult only streams the moving operand, so a second identical load is
    pure overhead (~53ns each). Only removes loads with no sync waits or
    updates and a static access pattern; tracking resets at block boundaries
    so hardware loops stay correct."""
    removed = 0
    for f in nc.m.functions:
        for b in f.blocks:
            last = None
            keep = []
            for inst in b.instructions:
                if isinstance(inst, mybir.InstLdweights):
                    a = inst.ins[0]
                    static = a.dynamic_ap_info is None and not a.regs_read()
                    sig = (
                        a.concise(), a.offset,
                        inst.perf_mode, inst.is_transpose, inst.tile_position,
                    ) if static else None
                    si = inst.sync_info
                    clean = not (si and (len(si.on_wait) or len(si.on_update)))
                    if sig is not None and sig == last and clean:
                        removed += 1
                        continue
                    last = sig
                keep.append(inst)
            if removed:
                b.instructions[:] = keep
    return removed


def _build_kernel(reps=1):
    f32 = mybir.dt.float32
    nc = bacc.Bacc(
        "TRN2",
        target_bir_lowering=False,
        debug=False,
        num_devices=N_CORES,
    )
    import contextlib

    xT = nc.declare_dram_parameter("xT", [IN_F, B_LOC], f32, isOutput=False)
    cw = nc.declare_dram_parameter("Cw", [N_KCHUNK * P, OUT_F], MM_DT, isOutput=False)
    xT_ap = xT[:, :].rearrange("(c p) b -> p c b", p=P)  # [128, 4, B_LOC]
    cw_ap = cw[:, :].rearrange("(k p) o -> p k o", p=P)  # [128, 32, 512]

    if LAYOUT == "cs":
        # bias broadcast tile: [p, os, b] with value bias[os*128+p]
        bias = nc.declare_dram_parameter("biasB", [P, N_OSLICE * 512], f32,
                                         isOutput=False)
        out = nc.declare_dram_parameter("outT", [OUT_F, B_LOC], f32, isOutput=True)
        out_ap = out[:, :].rearrange("(s p) b -> p s b", p=P)  # [128, 4, B_LOC]
        with tile.TileContext(nc) as tc:
            with (
                tc.tile_pool(name="const", bufs=1) as const_pool,
                tc.tile_pool(name="xin", bufs=3) as xin_pool,
                tc.tile_pool(name="tf32", bufs=2) as f32_pool,
                tc.tile_pool(name="cheb", bufs=6) as cheb_pool,
                tc.tile_pool(name="ot", bufs=4) as out_pool,
                tc.tile_pool(name="ps", bufs=1, space="PSUM") as psum_pool,
            ):
                c_tile = const_pool.tile([P, N_KCHUNK, OUT_F], MM_DT)
                nsplit = 4
                per = (N_KCHUNK + nsplit - 1) // nsplit
                for s in range(nsplit):
                    k0, k1 = s * per, min((s + 1) * per, N_KCHUNK)
                    nc.gpsimd.dma_start(out=c_tile[:, k0:k1, :], in_=cw_ap[:, k0:k1, :])
                b_tile = const_pool.tile([P, N_OSLICE, 512], f32)
                nc.gpsimd.dma_start(
                    out=b_tile[:, :, :],
                    in_=bias[:, :].rearrange("p (s b) -> p s b", s=N_OSLICE),
                )
                rep_ctx = (
                    tc.For_i(
                        0, reps, 1,
                        hint_engines=(
                            mybir.EngineType.PE,
                            mybir.EngineType.Activation,
                            mybir.EngineType.DVE,
                        ),
                    )
                    if reps > 1
                    else contextlib.nullcontext()
                )
                with rep_ctx:
                    _kernel_body_cs(nc, tc, xT_ap, c_tile, b_tile, out_ap,
                                    xin_pool, f32_pool, cheb_pool, out_pool,
                                    psum_pool)
    else:
        bias = nc.declare_dram_parameter("bias", [1, OUT_F], f32, isOutput=False)
        out = nc.declare_dram_parameter("out", [B_LOC, OUT_F], f32, isOutput=True)
        with tile.TileContext(nc) as tc:
            with (
                tc.tile_pool(name="const", bufs=1) as const_pool,
                tc.tile_pool(name="xin", bufs=3) as xin_pool,
                tc.tile_pool(name="tf32", bufs=1) as f32_pool,
                tc.tile_pool(name="cheb", bufs=2) as cheb_pool,
                tc.tile_pool(name="ot", bufs=4) as out_pool,
                tc.tile_pool(name="ps", bufs=6 // KSPLIT, space="PSUM") as psum_pool,
            ):
                c_tile = const_pool.tile([P, N_KCHUNK, OUT_F], MM_DT)
                nsplit = 4
                per = (N_KCHUNK + nsplit - 1) // nsplit
                for s in range(nsplit):
                    k0, k1 = s * per, min((s + 1) * per, N_KCHUNK)
                    nc.gpsimd.dma_start(out=c_tile[:, k0:k1, :], in_=cw_ap[:, k0:k1, :])
                # bias row (the folded d=0 term) broadcast to all 128 partitions
                b_tile = const_pool.tile([P, OUT_F], f32)
                bias_ap = bias[:, :]
                bias_bcast = bass.AP(
                    tensor=bias_ap.tensor,
                    offset=bias_ap.offset,
                    ap=[[0, P], bias_ap.ap[1]],
                )
                nc.gpsimd.dma_start(out=b_tile[:, :], in_=bias_bcast)

                rep_ctx = (
                    tc.For_i(
                        0, reps, 1,
                        hint_engines=(
                            mybir.EngineType.PE,
                            mybir.EngineType.Activation,
                            mybir.EngineType.DVE,
                        ),
                    )
                    if reps > 1
                    else contextlib.nullcontext()
                )
                with rep_ctx:
                    _kernel_body(nc, tc, xT_ap, c_tile, b_tile, out,
                                 xin_pool, f32_pool, cheb_pool, out_pool,
                                 psum_pool)
    nc.compile()
    if LAYOUT == "cs":
        n = _dedup_ldweights(nc)
        assert n > 0, "expected duplicate ldweights to remove"
    return nc


def _cheby_ops(nc):
    MULT = mybir.AluOpType.mult

    def stt(o, a, b):  # o = 2*a*b
        nc.vector.scalar_tensor_tensor(
            out=o, in0=a, scalar=2.0, in1=b, op0=MULT, op1=MULT
        )

    def sub1(o):  # o -= 1
        nc.vector.tensor_scalar(
            out=o, in0=o, scalar1=1.0, scalar2=None,
            op0=mybir.AluOpType.subtract,
        )

    return stt, sub1


def _kernel_body_cs(nc, tc, xT_ap, c_tile, b_tile, out_ap,
                    xin_pool, f32_pool, cheb_pool, out_pool, psum_pool):
    """C-stationary layout: psum[o-slice, b] accumulates over the 32 k-chunks;
    each LDWEIGHTS of C[k, os] feeds 2 N=512 matmuls (the two 512-col halves
    of the 1024-col b-group). Chunk order is ichunk-major to match the cheby
    production order, keeping PE fed from the first chain on."""
    f32 = mybir.dt.float32
    ACT_F = mybir.ActivationFunctionType
    MULT = mybir.AluOpType.mult
    stt, sub1 = _cheby_ops(nc)
    NBB = BBLK // 512  # 2

    for blk in range(B_LOC // BBLK):
        b0 = blk * BBLK
        tb_tiles = []
        for ic in range(N_ICHUNK):
            x_p = xin_pool.tile([P, BBLK], f32)
            nc.sync.dma_start(out=x_p[:, :], in_=xT_ap[:, ic, b0 : b0 + BBLK])
            tfs = f32_pool.tile([P, 4, BBLK], f32)
            t1, t2, t3, t4 = (tfs[:, j, :] for j in range(4))
            tb = cheb_pool.tile([P, DEG, BBLK], MM_DT)
            nc.scalar.activation(out=t1, in_=x_p[:, :], func=ACT_F.Tanh)
            nc.scalar.copy(out=tb[:, 0, :], in_=t1)
            stt(t2, t1, t1); sub1(t2)
            nc.scalar.copy(out=tb[:, 1, :], in_=t2)
            stt(t3, t2, t1); nc.vector.tensor_sub(t3, t3, t1)
            nc.scalar.copy(out=tb[:, 2, :], in_=t3)
            stt(t4, t2, t2); sub1(t4)
            nc.scalar.copy(out=tb[:, 3, :], in_=t4)
            b1, b2, b3, b4 = (tb[:, j, :] for j in range(4))
            b5, b6, b7, b8 = (tb[:, j, :] for j in range(4, 8))
            stt(b5, b3, b2); nc.vector.tensor_sub(b5, b5, b1)
            stt(b6, b3, b3); sub1(b6)
            stt(b7, b4, b3); nc.vector.tensor_sub(b7, b7, b1)
            stt(b8, b4, b4); sub1(b8)
            tb_tiles.append(tb)

        ps = [
            psum_pool.tile([P, 512], f32, space="PSUM",
                           tag=f"ps{os}_{bb}", name=f"ps{os}_{bb}")
            for os in range(N_OSLICE)
            for bb in range(NBB)
        ]
        for ic in range(N_ICHUNK):
            for j in range(DEG):
                k = j * N_ICHUNK + ic
                first = ic == 0 and j == 0
                last = ic == N_ICHUNK - 1 and j == DEG - 1
                for os in range(N_OSLICE):
                    for bb in range(NBB):
                        nc.tensor.matmul(
                            ps[os * NBB + bb][:, :],
                            c_tile[:, k, os * P : (os + 1) * P],
                            tb_tiles[ic][:, j, bb * 512 : (bb + 1) * 512],
                            start=first,
                            stop=last,
                        )
        for os in range(N_OSLICE):
            for bb in range(NBB):
                o_tile = out_pool.tile([P, 512], f32)
                # out = psum / C_SCALE + bias   (bias = sum_i C[i,:,0])
                nc.vector.scalar_tensor_tensor(
                    out=o_tile[:, :],
                    in0=ps[os * NBB + bb][:, :],
                    scalar=1.0 / C_SCALE,
                    in1=b_tile[:, os, :],
                    op0=MULT,
                    op1=mybir.AluOpType.add,
                )
                c0 = b0 + bb * 512
                nc.sync.dma_start(
                    out=out_ap[:, os, c0 : c0 + 512], in_=o_tile[:, :]
                )


def _kernel_body(nc, tc, xT_ap, c_tile, b_tile, out,
                 xin_pool, f32_pool, cheb_pool, out_pool, psum_pool):
    f32 = mybir.dt.float32
    MULT = mybir.AluOpType.mult
    ACT_F = mybir.ActivationFunctionType
    stt, sub1 = _cheby_ops(nc)

    for blk in range(B_LOC // BBLK):
        b0 = blk * BBLK
        x_in = xin_pool.tile([P, N_ICHUNK, BBLK], f32)
        nc.sync.dma_start(out=x_in[:, :, :], in_=xT_ap[:, :, b0 : b0 + BBLK])

        # Tf[:, j] = T_{j+1} in fp32 (j=0..3); Tb[:, j] = T_{j+1} in bf16 (j=0..7)
        Tf = f32_pool.tile([P, 4, N_ICHUNK, BBLK], f32)
        Tb = cheb_pool.tile([P, DEG, N_ICHUNK, BBLK], MM_DT)
        t1, t2, t3, t4 = (Tf[:, j, :, :] for j in range(4))
        nc.scalar.activation(out=t1, in_=x_in[:, :, :], func=ACT_F.Tanh)
        # fp32 chain: T2=2T1^2-1, T3=2T2T1-T1, T4=2T2^2-1
        stt(t2, t1, t1); sub1(t2)
        stt(t3, t2, t1); nc.vector.tensor_sub(t3, t3, t1)
        stt(t4, t2, t2); sub1(t4)
        # one-time rounding to bf16 on the scalar engine
        for j in range(4):
            nc.scalar.copy(out=Tb[:, j, :, :], in_=Tf[:, j, :, :])
        b1, b2, b3, b4 = (Tb[:, j, :, :] for j in range(4))
        b5, b6, b7, b8 = (Tb[:, j, :, :] for j in range(4, 8))
        # bf16 products: T5=2T3T2-T1, T6=2T3^2-1, T7=2T4T3-T1, T8=2T4^2-1
        stt(b5, b3, b2); nc.vector.tensor_sub(b5, b5, b1)
        stt(b6, b3, b3); sub1(b6)
        stt(b7, b4, b3); nc.vector.tensor_sub(b7, b7, b1)
        stt(b8, b4, b4); sub1(b8)

        for bt in range(BBLK // P):
            H = P // KSPLIT
            halves = [(h, h * H) for h in range(KSPLIT)]
            ps = [
                psum_pool.tile(
                    [P, OUT_F], f32, space="PSUM", tag=f"ps{h}", name=f"ps{h}"
                )
                for h in range(KSPLIT)
            ]
            bsl = slice(bt * P, (bt + 1) * P)
            for j in range(DEG):
                for c in range(N_ICHUNK):
                    k = j * N_ICHUNK + c
                    for h, lo in halves:
                        nc.tensor.matmul(
                            ps[h][:, :],
                            Tb[lo : lo + H, j, c, bsl],
                            c_tile[lo : lo + H, k, :],
                            start=(k == 0),
                            stop=(k == N_KCHUNK - 1),
                        )
            o_tile = out_pool.tile([P, OUT_F], f32)
            row = b0 + bt * P
            acc = ps[0][:, :]
            if KSPLIT > 1:
                half_sb = out_pool.tile([P, OUT_F], f32, tag="halfsb")
                nc.scalar.copy(out=half_sb[:, :], in_=ps[0][:, :])
                for h in range(1, KSPLIT - 1):
                    nc.vector.tensor_add(half_sb[:, :], half_sb[:, :], ps[h][:, :])
                nc.vector.tensor_add(half_sb[:, :], half_sb[:, :], ps[KSPLIT - 1][:, :])
                acc = half_sb[:, :]
            # out = psum / C_SCALE + bias   (bias = sum_i C[i,:,0], the d=0 term)
            nc.vector.scalar_tensor_tensor(
                out=o_tile[:, :],
                in0=acc,
                scalar=1.0 / C_SCALE,
                in1=b_tile[:, :],
                op0=MULT,
                op1=mybir.AluOpType.add,
            )
            nc.sync.dma_start(out=out[row : row + P, :], in_=o_tile[:, :])


@lru_cache(maxsize=4)
def _get_nc(reps=1):
    return _build_kernel(reps)


class Runner:
    """Persistent jitted runner mirroring bass2jax.run_bass_via_pjrt, reusable
    across calls (single jit cache entry) so repeated executions can be timed
    back-to-back without recompilation or host round-trips per call."""

    def __init__(self, nc):
        import jax
        import jax.numpy as jnp
        from jax.sharding import Mesh, PartitionSpec
        from jax.experimental.shard_map import shard_map
        from concourse import bass2jax
        from concourse import mybir as _mybir

        bass2jax.install_neuronx_cc_hook()
        self.jax = jax
        self.nc = nc
        partition_name = (
            nc.partition_id_tensor.name if nc.partition_id_tensor else None
        )
        in_names, out_names, out_avals = [], [], []
        for alloc in nc.m.functions[0].allocations:
            if not isinstance(alloc, _mybir.MemoryLocationSet):
                continue
            name = alloc.memorylocations[0].name
            if alloc.kind == "ExternalInput":
                if name != partition_name:
                    in_names.append(name)
            elif alloc.kind == "ExternalOutput":
                out_names.append(name)
                out_avals.append(
                    jax.core.ShapedArray(
                        tuple(alloc.tensor_shape), _mybir.dt.np(alloc.dtype)
                    )
                )
        self.in_names = list(in_names)
        self.out_names = out_names
        self.out_avals = out_avals
        n_params = len(in_names)
        all_names = in_names + out_names
        if partition_name is not None:
            all_names = all_names + [partition_name]

        def _body(*args):
            operands = list(args)
            if partition_name is not None:
                operands.append(bass2jax.partition_id_tensor())
            return tuple(
                bass2jax._bass_exec_p.bind(
                    *operands,
                    out_avals=tuple(out_avals),
                    in_names=tuple(all_names),
                    out_names=tuple(out_names),
                    lowering_input_output_aliases=(),
                    sim_require_finite=True,
                    sim_require_nnan=True,
                    nc=nc,
                )
            )

        devices = jax.devices()[:N_CORES]
        self.mesh = Mesh(np.asarray(devices), ("core",))
        in_specs = (PartitionSpec("core"),) * (n_params + len(out_names))
        out_specs = (PartitionSpec("core"),) * len(out_names)
        self.fn = jax.jit(
            shard_map(
                _body,
                mesh=self.mesh,
                in_specs=in_specs,
                out_specs=out_specs,
                check_rep=False,
            ),
            keep_unused=True,
        )

    def put_inputs(self, in_maps):
        import jax
        from jax.sharding import NamedSharding, PartitionSpec

        concat = [
            np.concatenate([np.asarray(m[name]) for m in in_maps], axis=0)
            for name in self.in_names
        ]
        for aval in self.out_avals:
            concat.append(
                np.zeros((N_CORES * aval.shape[0], *aval.shape[1:]), aval.dtype)
            )
        sh = NamedSharding(self.mesh, PartitionSpec("core"))
        return [jax.device_put(a, sh) for a in concat]

    def __call__(self, dev_inputs):
        return self.fn(*dev_inputs)

    def run_np(self, in_maps):
        outs = self(self.put_inputs(in_maps))
        return [
            {
                name: np.asarray(outs[i]).reshape(N_CORES, *self.out_avals[i].shape)[c]
                for i, name in enumerate(self.out_names)
            }
            for c in range(N_CORES)
        ]


def _prep_inputs(x: np.ndarray, coefficients: np.ndarray):
    x = np.asarray(x, dtype=np.float32)
    coefficients = np.asarray(coefficients, dtype=np.float32)
    # chunk k = j*4+c is degree j+1, i-chunk c, laid out [i within chunk, o];
    # the d=0 term (T_0 == 1) reduces to a bias row added at copy-out.
    c_perm = np.transpose(coefficients, (2, 0, 1))  # (d, i, o)
    bias = np.ascontiguousarray(c_perm[0].sum(axis=0, dtype=np.float64))
    bias = bias.astype(np.float32)
    c_main = c_perm[1:].reshape(N_KCHUNK * P, OUT_F) * C_SCALE
    c_all = np.ascontiguousarray(c_main).astype(MM_NP)

    if LAYOUT == "cs":
        # biasB[p, os*512 + b] = bias[os*128 + p]  (broadcast along b)
        bias_t = bias.reshape(N_OSLICE, P).T  # [128, 4]
        bias_b = np.ascontiguousarray(
            np.broadcast_to(bias_t[:, :, None], (P, N_OSLICE, 512))
        ).reshape(P, N_OSLICE * 512)
    else:
        bias_b = bias.reshape(1, OUT_F)

    in_maps = []
    for core in range(N_CORES):
        shard = x[core * B_LOC : (core + 1) * B_LOC]  # (4096, 512)
        xt = np.ascontiguousarray(shard.T)  # (512, 4096)
        m = {"xT": xt, "Cw": c_all}
        m["biasB" if LAYOUT == "cs" else "bias"] = bias_b
        in_maps.append(m)
    return in_maps


@lru_cache(maxsize=4)
def _get_runner(reps=1):
    return Runner(_get_nc(reps))


def run_sharded(x, coefficients):
    """Run the 8-core kernel; returns the full (32768, 512) float32 output."""
    in_maps = _prep_inputs(x, coefficients)
    runner = _get_runner()
    results = runner.run_np(in_maps)
    if LAYOUT == "cs":
        parts = [np.asarray(results[i]["outT"]).T for i in range(N_CORES)]
    else:
        parts = [np.asarray(results[i]["out"]) for i in range(N_CORES)]
    return np.concatenate(parts, axis=0).astype(np.float32)


def _time_runner(runner, dev_in, iters):
    import time

    outs = runner(dev_in)  # warm up
    outs[0].block_until_ready()
    times = []
    for _ in range(iters):
        t0 = time.perf_counter()
        outs = runner(dev_in)
        outs[0].block_until_ready()
        times.append((time.perf_counter() - t0) * 1e9)
    return times


def bench(x, coefficients, iters=12, rep_a=3, rep_b=83):
    """Estimate per-invocation HW time from the slope between two on-device
    repeat counts (fixed ~66-107ms axon RPC overhead cancels). Interleaved
    rounds + median to reject the bimodal RPC jitter. Returns
    (slope_ns, times_a, times_b)."""
    in_maps = _prep_inputs(x, coefficients)
    ra, rb = _get_runner(rep_a), _get_runner(rep_b)
    dev_a = ra.put_inputs(in_maps)
    dev_b = rb.put_inputs(in_maps)
    ta, tb = [], []
    for _ in range(3):
        ta += _time_runner(ra, dev_a, iters // 3 + 1)
        tb += _time_runner(rb, dev_b, iters // 3 + 1)
    med = lambda t: sorted(t)[len(t) // 2]
    slope = (med(tb) - med(ta)) / (rep_b - rep_a)
    return slope, ta, tb


def kernel(x, coefficients):
    return run_sharded(x, coefficients)
